# revision 49
# baseline (speedup 1.0000x reference)
"""Trainium2 Bass kernel for nn_MinamoScoreHead (vision conv head + GCN topo head).

Sharding: data-parallel over 8 NeuronCores. Each core gets 8 images (vision
head) and 8 whole graphs (topo head: nodes + all edges whose dst lies in
those graphs). Small weights are replicated. Everything heavy runs on device:
 - 3x3 valid conv in fp8e4m3: taps paired into DoubleRow matmuls (2x PE rate),
   weights pre-scaled by WS to dodge fp8 subnormals; epilogue folds 1/WS +
   bias + leaky-relu into one Prelu activation.
 - adaptive max-pool via vector tensor_reduce (max commutes with the monotone
   epilogue)
 - GCN aggregation: dma_gather of (topo * dis)[src] rows by deduped
   (window, src) key over 4 SWDGE queues (the single-queue gather was the
   original bottleneck: queue serialization, not HBM bandwidth); segment-sum
   over sorted edges as matmul with fp8 count matrices S (exact small
   integers) into per-window PSUM; dis[dst] is applied post-matmul via a
   broadcast multiply, so no precision is lost to fp8 in S.
 - self-loops never touch the gather: per-window identity matmuls over
   sequentially-loaded own-node rows
 - graphs are assigned to cores by size rank so per-window chunk counts
   (max across cores) carry minimal padding
 - gcn W + bias/dis (+ padded-slot poison) via matmuls, Prelu, dis scale,
   per-graph max as windows complete
 - spectral-norm scale factors are folded into the weights on host (cheap
   O(D^2) scalar math, identical to the reference power iteration)
"""
import os
import numpy as np
import ml_dtypes

from concourse import bacc, mybir
from concourse.tile import TileContext
from concourse.bass_utils import run_bass_kernel_spmd
from concourse.ap import AP

BF16 = ml_dtypes.bfloat16
E4M3 = ml_dtypes.float8_e4m3fn

# problem constants
N_NODES = 20000
N_EDGES = 640000
D = 128
OUT = 256
B = 64
HW = 64
NEG = 0.2

NCORES = 8
IMG_PER_CORE = B // NCORES          # 8
G_PER_CORE = B // NCORES            # 8
P_G = 512                           # slots per graph
NSLOT = G_PER_CORE * P_G            # 4096
NWIN = NSLOT // 128                 # 32
CHUNK = 128                         # gather rows per scatter-matmul
CALL_CHUNKS = 8                     # chunks per dma_gather call (1024 idx: SWDGE ring cap)
NQ = 4                              # SWDGE queues (ucode max 4)
XCOLS = HW * HW + 4                 # padded image row buffer (4100)
WS = 16.0                           # conv fp8 weight pre-scale

# conv tap pairing for DoubleRow: ((dh1,dw1),(dh2,dw2)|None), rhs delta
PAIRS = [((0, 0), (1, 0)), ((0, 1), (1, 1)), ((0, 2), (1, 2)),
         ((2, 0), (2, 1)), ((2, 2), None)]
PAIR_DELTA = [64, 64, 64, 1, 1]

LAST_EXEC_NS = None
LAST_RESULT = None


def _sn_scale(w2d, u):
    """Spectral-norm 1/sigma, mimicking the reference power iteration (f32)."""
    w2d = w2d.astype(np.float32)
    u = u.astype(np.float32)
    v = w2d.T @ u
    v = v / (np.linalg.norm(v) + 1e-12)
    u2 = w2d @ v
    u2 = u2 / (np.linalg.norm(u2) + 1e-12)
    sigma = u2 @ (w2d @ v)
    return np.float32(1.0) / sigma


def _conv_blocks():
    """Row blocks of the 62-row conv output, aligned to the 31-row pool halves."""
    blocks = []
    for ph, r0 in ((0, 0), (1, 31)):
        for k in range(4):
            i0 = r0 + 8 * k
            R = 8 if k < 3 else 7
            blocks.append((i0, R, ph))
    return blocks


def _build_schedule(inputs):
    """All host-side preprocessing: shard, dedup+sort edges, build S/idx arrays."""
    vis = np.asarray(inputs["vis"], dtype=np.float32)
    topo = np.asarray(inputs["topo"], dtype=np.float32)
    edge_index = np.asarray(inputs["edge_index"], dtype=np.int64)
    batch = np.asarray(inputs["batch"], dtype=np.int64)

    conv_w = np.asarray(inputs["conv_w"], dtype=np.float32)
    conv_b = np.asarray(inputs["conv_b"], dtype=np.float32)
    fcv_w = np.asarray(inputs["fcv_w"], dtype=np.float32)
    fcv_b = np.asarray(inputs["fcv_b"], dtype=np.float32)
    gcn_w = np.asarray(inputs["gcn_w"], dtype=np.float32)
    gcn_b = np.asarray(inputs["gcn_b"], dtype=np.float32)
    fct_w = np.asarray(inputs["fct_w"], dtype=np.float32)
    fct_b = np.asarray(inputs["fct_b"], dtype=np.float32)

    # ---- spectral norm folded into weights
    s_conv = _sn_scale(conv_w.reshape(D, -1), np.asarray(inputs["conv_u"]))
    s_fcv = _sn_scale(fcv_w, np.asarray(inputs["fcv_u"]))
    s_fct = _sn_scale(fct_w, np.asarray(inputs["fct_u"]))
    conv_ws = conv_w * s_conv
    fcv_ws = fcv_w * s_fcv
    fct_ws = fct_w * s_fct

    # ---- graph structure
    counts = np.bincount(batch, minlength=B)
    assert counts.max() <= P_G, f"graph too large: {counts.max()}"
    assert counts.min() > 0, "empty graph unsupported"
    starts = np.zeros(B + 1, dtype=np.int64)
    np.cumsum(counts, out=starts[1:])
    nodes = np.arange(N_NODES, dtype=np.int64)

    # balance graphs across cores: rank by size, graph ranked[r] -> core r%8,
    # position r//8 -- so each core's position-p graph has a similar size and
    # the max-across-cores chunk padding nearly vanishes
    ranked = np.argsort(-counts, kind="stable")
    core_of_graph = np.zeros(B, dtype=np.int64)
    pos_of_graph = np.zeros(B, dtype=np.int64)
    core_of_graph[ranked] = np.arange(B) % NCORES
    pos_of_graph[ranked] = np.arange(B) // NCORES

    slot_node = pos_of_graph[batch] * P_G + (nodes - starts[batch])

    deg = (1.0 + np.bincount(edge_index[1], minlength=N_NODES)).astype(np.float32)
    dis = (1.0 / np.sqrt(deg)).astype(np.float32)

    # self-loops are handled by per-window identity matmuls, not gathered
    src_all = edge_index[0]
    dst_all = edge_index[1]
    core_all = core_of_graph[batch[dst_all]]
    dslot_all = slot_node[dst_all]

    # dis[src] folded into the gather table; dis[dst] applied post-aggregation
    table = np.ascontiguousarray((topo * dis[:, None]).astype(BF16))

    per_core = []
    win_counts = np.zeros((NCORES, NWIN), dtype=np.int64)   # unique rows per win
    for c in range(NCORES):
        sel = core_all == c
        src_c = src_all[sel]
        dslot_c = dslot_all[sel]
        win_c = dslot_c // 128
        # dedup gather rows on (window, src); S entries become edge counts
        key = win_c * N_NODES + src_c
        uk, inv = np.unique(key, return_inverse=True)
        g_win = uk // N_NODES
        g_src = uk % N_NODES
        win_counts[c] = np.bincount(g_win, minlength=NWIN)
        per_core.append((g_src, g_win, inv, dslot_c, win_c))

    c_w = (win_counts.max(axis=0) + CHUNK - 1) // CHUNK      # chunks per window
    # note: streaming windows in descending-size order (to shorten the tail)
    # was tried and crashes the neuron runtime; keep natural order
    win_chunk_base = np.zeros(NWIN, dtype=np.int64)
    win_chunk_base[1:] = np.cumsum(c_w)[:-1]
    t_chunks = int(c_w.sum())
    e_pad = t_chunks * CHUNK

    # gather calls: full-size except the tail, which uses small calls so the
    # final ring drain is short
    call_sizes = []
    rem = t_chunks
    tail_chunks = min(16, max(0, rem - CALL_CHUNKS))
    while rem > tail_chunks:
        k = min(CALL_CHUNKS, rem - tail_chunks)
        call_sizes.append(k)
        rem -= k
    while rem > 0:
        k = min(4, rem)
        call_sizes.append(k)
        rem -= k

    in_maps = []
    vis_f8 = vis.reshape(B, D, HW * HW).astype(E4M3)

    # ---- replicated weights
    # conv pairs: W2[cin, p, i, cout] = conv_ws[cout, cin, dh_i, dw_i] * WS
    W2 = np.zeros((D, len(PAIRS), 2, D), dtype=np.float32)
    for p, (t1, t2) in enumerate(PAIRS):
        for i, t in enumerate((t1, t2)):
            if t is None:
                continue
            dh, dw = t
            W2[:, p, i, :] = conv_ws[:, :, dh, dw].T * WS
    W2 = np.ascontiguousarray(W2.reshape(D, len(PAIRS) * 2 * D)).astype(E4M3)
    conv_bias = conv_b.reshape(D, 1).astype(np.float32)
    gcnW = gcn_w.astype(BF16)                                   # [d_in, d_out]
    biasp = np.stack([gcn_b, np.ones(D, np.float32)]).astype(BF16)   # [2, 128]
    # fcvW[c, q*256+o] = fcv_ws[o, c*4+q]
    fcvW = np.ascontiguousarray(
        fcv_ws.reshape(OUT, D, 4).transpose(1, 2, 0).reshape(D, 4 * OUT)).astype(BF16)
    fcv_brow = fcv_b.reshape(1, OUT).astype(BF16)
    fctW = np.ascontiguousarray(fct_ws.T).astype(BF16)          # [128, 256]
    fct_brow = fct_b.reshape(1, OUT).astype(BF16)

    I8 = np.eye(CHUNK, dtype=np.float32).astype(E4M3)

    for c in range(NCORES):
        g_src, g_win, inv, dslot_c, win_c = per_core[c]
        # row position for each unique (win, src) group: groups are sorted by
        # (win, src), so rank within window = global rank - window start rank
        ng = g_src.shape[0]
        rank = np.arange(ng, dtype=np.int64)
        gw_start = np.searchsorted(g_win, np.arange(NWIN))
        rowpos = win_chunk_base[g_win] * CHUNK + (rank - gw_start[g_win])

        srcp = np.zeros(e_pad, dtype=np.int64)
        srcp[rowpos] = g_src

        # S: [128, t_chunks*128] fp8; S[p, t*128+dcol] = count
        S = np.zeros((CHUNK, t_chunks * CHUNK), dtype=np.float32)
        erow = rowpos[inv]                       # padded row of each edge
        t_arr = erow // CHUNK
        p_arr = erow % CHUNK
        dcol = dslot_c - win_c * 128
        np.add.at(S, (p_arr, t_arr * CHUNK + dcol), 1.0)
        S8 = S.astype(E4M3)

        # gather index wrapping: idx j -> [16m + j%16, j//16], m=0..7
        base16 = srcp.reshape(-1, 16).T.astype(np.int16)         # [16, e_pad/16]
        gidx = np.ascontiguousarray(np.tile(base16, (8, 1)))     # [128, e_pad/16]

        # self-loop rows: window w holds rows [i] = table[start_g + 128*(w%4) + i]
        # (zero-padded past the graph); consumed by identity matmuls on device
        xwall = np.zeros((CHUNK, NWIN, D), dtype=BF16)
        graphs_c = ranked[np.arange(G_PER_CORE) * NCORES + c]
        for w in range(NWIN):
            gid = graphs_c[w // 4]
            r_w = int(np.clip(counts[gid] - 128 * (w % 4), 0, 128))
            if r_w > 0:
                n0 = starts[gid] + 128 * (w % 4)
                xwall[:r_w, w, :] = table[n0:n0 + r_w]

        # per-slot dis (for post-scale) and bias/poison rows
        gslots = np.arange(NSLOT, dtype=np.int64)
        gg = graphs_c[gslots // P_G]
        local = gslots % P_G
        node_of_slot = starts[gg] + local
        real = local < counts[gg]
        dis_slot = np.where(real, dis[np.minimum(node_of_slot, N_NODES - 1)], 1.0)
        dis_slot = dis_slot.astype(np.float32)
        disB = np.ascontiguousarray(
            np.broadcast_to(dis_slot.astype(BF16), (D, NSLOT)))
        maskb2 = np.zeros((2, NSLOT), dtype=np.float32)
        maskb2[0] = np.where(real, 1.0 / dis_slot, 0.0)
        maskb2[1] = np.where(real, 0.0, -1.0e9)
        maskb2 = maskb2.astype(BF16)

        in_maps.append({
            "vis": np.ascontiguousarray(vis_f8[c * IMG_PER_CORE:(c + 1) * IMG_PER_CORE]),
            "topo": table,
            "gidx": gidx,
            "S": S8,
            "XW": np.ascontiguousarray(xwall.reshape(CHUNK, NWIN * D)),
            "I8": I8,
            "disB": disB,
            "maskb2": maskb2,
            "convW2": W2,
            "conv_bias": conv_bias,
            "gcnW": gcnW,
            "biasp": biasp,
            "fcvW": fcvW,
            "fcv_brow": fcv_brow,
            "fctW": fctW,
            "fct_brow": fct_brow,
        })

    sched = dict(t_chunks=t_chunks, c_w=[int(x) for x in c_w],
                 win_start=[int(x) for x in win_chunk_base],
                 call_sizes=call_sizes,
                 core_of_graph=core_of_graph, pos_of_graph=pos_of_graph)
    return in_maps, sched


def _pair_rhs(xap, base, delta, n):
    """[128, 2, n] view of a [128, C] SBUF tile: slots at base and base+delta."""
    a = xap[:, base:base + n]
    return AP(a.tensor, a.offset, [list(a.ap[0]), [delta, 2], list(a.ap[1])])


def _build_program(t_chunks, c_w, win_start, call_sizes):
    nc = bacc.Bacc(None, target_bir_lowering=False, num_swdge_queues=NQ)
    f32 = mybir.dt.float32
    bf16 = mybir.dt.bfloat16
    fp8 = mybir.dt.float8e4

    vis_d = nc.declare_dram_parameter("vis", [IMG_PER_CORE, D, HW * HW], fp8, isOutput=False)
    topo_d = nc.declare_dram_parameter("topo", [N_NODES, D], bf16, isOutput=False)
    gidx_d = nc.declare_dram_parameter("gidx", [128, (t_chunks * CHUNK) // 16], mybir.dt.int16, isOutput=False)
    S_d = nc.declare_dram_parameter("S", [128, t_chunks * CHUNK], fp8, isOutput=False)
    XW_d = nc.declare_dram_parameter("XW", [CHUNK, NWIN * D], bf16, isOutput=False)
    I8_d = nc.declare_dram_parameter("I8", [CHUNK, CHUNK], fp8, isOutput=False)
    disB_d = nc.declare_dram_parameter("disB", [D, NSLOT], bf16, isOutput=False)
    maskb2_d = nc.declare_dram_parameter("maskb2", [2, NSLOT], bf16, isOutput=False)
    convW2_d = nc.declare_dram_parameter("convW2", [D, len(PAIRS) * 2 * D], fp8, isOutput=False)
    conv_bias_d = nc.declare_dram_parameter("conv_bias", [D, 1], f32, isOutput=False)
    gcnW_d = nc.declare_dram_parameter("gcnW", [D, D], bf16, isOutput=False)
    biasp_d = nc.declare_dram_parameter("biasp", [2, D], bf16, isOutput=False)
    fcvW_d = nc.declare_dram_parameter("fcvW", [D, 4 * OUT], bf16, isOutput=False)
    fcv_brow_d = nc.declare_dram_parameter("fcv_brow", [1, OUT], bf16, isOutput=False)
    fctW_d = nc.declare_dram_parameter("fctW", [D, OUT], bf16, isOutput=False)
    fct_brow_d = nc.declare_dram_parameter("fct_brow", [1, OUT], bf16, isOutput=False)

    vis_out_d = nc.declare_dram_parameter("vis_out", [IMG_PER_CORE, OUT], f32, isOutput=True)
    topo_out_d = nc.declare_dram_parameter("topo_out", [G_PER_CORE, OUT], f32, isOutput=True)

    ncalls = len(call_sizes)
    call_base = np.zeros(ncalls + 1, dtype=np.int64)
    np.cumsum(call_sizes, out=call_base[1:])

    # window -> last gather call it needs
    def last_call(w):
        if c_w[w] == 0:
            return -1
        last_chunk = win_start[w] + c_w[w] - 1
        return int(np.searchsorted(call_base[1:], last_chunk, side="right"))

    blocks = _conv_blocks()
    CP = mybir.ActivationFunctionType.Copy
    PR = mybir.ActivationFunctionType.Prelu
    DR = mybir.MatmulPerfMode.DoubleRow

    with TileContext(nc) as tc:
        with tc.tile_pool(name="const", bufs=1) as cpool, \
             tc.tile_pool(name="xin", bufs=2) as xpool, \
             tc.tile_pool(name="gat", bufs=14) as gpool, \
             tc.tile_pool(name="spool", bufs=14) as spool, \
             tc.tile_pool(name="small", bufs=4) as smpool, \
             tc.tile_pool(name="cps", bufs=2, space="PSUM") as conv_ps, \
             tc.tile_pool(name="aps", bufs=3, space="PSUM") as agg_ps, \
             tc.tile_pool(name="hps", bufs=2, space="PSUM") as h_ps, \
             tc.tile_pool(name="fps", bufs=1, space="PSUM") as fc_ps:

            # ---- constants (gidx head first: it gates the gather stream)
            all_cols = (t_chunks * CHUNK) // 16
            head_calls = min(NQ + 3, ncalls)
            head_cols = min(int(call_base[head_calls]) * 8, all_cols)
            gidx_h = cpool.tile([128, head_cols], mybir.dt.int16)
            nc.sync.dma_start(out=gidx_h[:], in_=gidx_d[:, :head_cols])
            # prime the 4 SWDGE rings with tiny gathers (128 real indices each)
            # so the first-call ring-warmup cost overlaps the const loads
            warm = cpool.tile([128, NQ, CHUNK], bf16)
            for q in range(NQ):
                nc.gpsimd.dma_gather(
                    out_ap=warm[:, q:q + 1, :],
                    in_ap=topo_d[:],
                    idxs_ap=gidx_h[:, :8],
                    num_idxs=CHUNK,
                    num_idxs_reg=CHUNK,
                    elem_size=D,
                    queue_num=q,
                )
            gidx_r = None
            if head_cols < all_cols:
                gidx_r = cpool.tile([128, all_cols - head_cols], mybir.dt.int16)
                nc.sync.dma_start(out=gidx_r[:], in_=gidx_d[:, head_cols:])
            convW2 = cpool.tile([D, len(PAIRS), 2, D], fp8)
            conv_bias = cpool.tile([D, 1], f32)
            gcnW = cpool.tile([D, D], bf16)
            biasp = cpool.tile([2, D], bf16)
            maskb2 = cpool.tile([2, NSLOT], bf16)
            disB = cpool.tile([D, NSLOT], bf16)
            fcvW = cpool.tile([D, 4 * OUT], bf16)
            fcv_brow = cpool.tile([1, OUT], bf16)
            fctW = cpool.tile([D, OUT], bf16)
            fct_brow = cpool.tile([1, OUT], bf16)
            xwall = cpool.tile([CHUNK, NWIN, D], bf16)
            I8 = cpool.tile([CHUNK, CHUNK], fp8)

            def load_consts():
                """Emitted after the first gather calls so the small gidx head
                DMA reaches the front of the sync queue."""
                nc.sync.dma_start(out=convW2[:], in_=convW2_d[:].rearrange(
                    "c (p i o) -> c p i o", p=len(PAIRS), i=2))
                nc.sync.dma_start(out=conv_bias[:], in_=conv_bias_d[:])
                nc.sync.dma_start(out=gcnW[:], in_=gcnW_d[:])
                nc.sync.dma_start(out=biasp[:], in_=biasp_d[:])
                nc.sync.dma_start(out=maskb2[:], in_=maskb2_d[:])
                nc.sync.dma_start(out=disB[:], in_=disB_d[:])
                nc.sync.dma_start(out=fcvW[:], in_=fcvW_d[:])
                nc.sync.dma_start(out=fcv_brow[:], in_=fcv_brow_d[:])
                nc.sync.dma_start(out=fctW[:], in_=fctW_d[:])
                nc.sync.dma_start(out=fct_brow[:], in_=fct_brow_d[:])
                nc.sync.dma_start(out=xwall[:], in_=XW_d[:].rearrange(
                    "p (w d) -> p w d", w=NWIN))
                nc.sync.dma_start(out=I8[:], in_=I8_d[:])

            acc_all = cpool.tile([D, IMG_PER_CORE * 4], f32)
            nc.vector.memset(acc_all[:], -3.0e38)
            ones1 = cpool.tile([1, max(IMG_PER_CORE, G_PER_CORE)], bf16)
            nc.vector.memset(ones1[:], 1.0)
            aggT = cpool.tile([D, NSLOT], bf16)
            hT = cpool.tile([D, NSLOT], bf16)

            gtiles = {}
            stiles = {}
            pooled = cpool.tile([D, G_PER_CORE], f32)
            wins_done = [0] * G_PER_CORE

            def note_window_done(w):
                gidx_ = w // 4
                wins_done[gidx_] += 1
                if wins_done[gidx_] == 4:
                    nc.vector.tensor_reduce(
                        out=pooled[:, gidx_:gidx_ + 1],
                        in_=hT[:, gidx_ * P_G:(gidx_ + 1) * P_G],
                        axis=mybir.AxisListType.X, op=mybir.AluOpType.max)

            def emit_call(k):
                nchunk = call_sizes[k]
                nidx = nchunk * CHUNK
                g = gpool.tile([128, CALL_CHUNKS, CHUNK], bf16, tag="gat")
                c0, c1 = int(call_base[k]) * 8, int(call_base[k + 1]) * 8
                if k < head_calls:
                    idxs = gidx_h[:, c0:c1]
                else:
                    idxs = gidx_r[:, c0 - head_cols:c1 - head_cols]
                nc.gpsimd.dma_gather(
                    out_ap=g[:, :nchunk, :],
                    in_ap=topo_d[:],
                    idxs_ap=idxs,
                    num_idxs=nidx,
                    num_idxs_reg=nidx,
                    elem_size=D,
                    queue_num=k % NQ,
                )
                s = spool.tile([128, CALL_CHUNKS * CHUNK], fp8, tag="spool")
                eng = nc.sync
                eng.dma_start(
                    out=s[:, :nchunk * CHUNK],
                    in_=S_d[:, int(call_base[k]) * CHUNK: int(call_base[k + 1]) * CHUNK],
                )
                gtiles[k] = g
                stiles[k] = s

            def emit_window(w):
                agg = agg_ps.tile([D, 128], f32, tag="aps")
                # self-loops: agg[:, slot] += XW[slot, :] (zero rows past graph)
                nc.tensor.matmul(out=agg[:], lhsT=xwall[:, w, :], rhs=I8[:],
                                 start=True, stop=(c_w[w] == 0))
                wend = win_start[w] + c_w[w]
                for t in range(int(win_start[w]), int(wend)):
                    k = int(np.searchsorted(call_base[1:], t, side="right"))
                    off = t - int(call_base[k])
                    nc.tensor.matmul(
                        out=agg[:],
                        lhsT=gtiles[k][:, off, :],
                        rhs=stiles[k][:, off * CHUNK:(off + 1) * CHUNK],
                        start=False, stop=(t == int(wend) - 1),
                    )
                nc.scalar.activation(out=aggT[:, w * 128:(w + 1) * 128], in_=agg[:], func=CP)
                h = h_ps.tile([D, 128], f32, tag="hps")
                nc.tensor.matmul(out=h[:], lhsT=gcnW[:], rhs=aggT[:, w * 128:(w + 1) * 128],
                                 start=True, stop=False)
                nc.tensor.matmul(out=h[:], lhsT=biasp[:], rhs=maskb2[:, w * 128:(w + 1) * 128],
                                 start=False, stop=True)
                # leaky(h) then * dis[dst]  (leaky(x*d) = leaky(x)*d for d>0)
                hl = smpool.tile([D, 128], bf16, tag="hl")
                nc.scalar.activation(out=hl[:], in_=h[:], func=PR, alpha=NEG)
                nc.vector.tensor_tensor(out=hT[:, w * 128:(w + 1) * 128],
                                        in0=hl[:], in1=disB[:, w * 128:(w + 1) * 128],
                                        op=mybir.AluOpType.mult)
                note_window_done(w)

            def emit_conv(img):
                x = xpool.tile([D, XCOLS], fp8, tag="xin")
                nc.sync.dma_start(out=x[:, :HW * HW], in_=vis_d[img])
                nc.vector.memset(x[:, HW * HW:], 0.0)
                xap = x[:]
                for (i0, R, ph2) in blocks:
                    n = R * HW
                    ps = conv_ps.tile([D, 512], f32, tag="cps")
                    for p in range(len(PAIRS)):
                        (dh1, dw1) = PAIRS[p][0]
                        base = (i0 + dh1) * HW + dw1
                        nc.tensor.matmul(
                            out=ps[:, :n],
                            lhsT=convW2[:, p, :, :],
                            rhs=_pair_rhs(xap, base, PAIR_DELTA[p], n),
                            start=(p == 0), stop=(p == len(PAIRS) - 1),
                            perf_mode=DR,
                        )
                    red = smpool.tile([D, 2], f32, tag="red")
                    ap = ps[:, :n].rearrange("p (r c) -> p r c", c=HW)[:, :, :62] \
                                  .rearrange("p r (q w) -> p q r w", q=2)
                    nc.vector.tensor_reduce(out=red[:], in_=ap, axis=mybir.AxisListType.XY,
                                            op=mybir.AluOpType.max)
                    accs = acc_all[:, img * 4 + ph2 * 2: img * 4 + ph2 * 2 + 2]
                    nc.vector.tensor_tensor(out=accs, in0=accs, in1=red[:],
                                            op=mybir.AluOpType.max)

            # ---- emission schedule: interleave conv images, gather calls, windows
            win_of_call = [[] for _ in range(ncalls)]
            for w in range(NWIN):
                lc = last_call(w)
                if lc >= 0:
                    win_of_call[lc].append(w)
            empty_wins = [w for w in range(NWIN) if c_w[w] == 0]

            emitted_calls = 0

            def ensure_calls(upto):
                nonlocal emitted_calls
                while emitted_calls <= min(upto, ncalls - 1):
                    emit_call(emitted_calls)
                    emitted_calls += 1

            load_consts()
            ensure_calls(NQ + 2)     # prefetch first calls
            for w in empty_wins:
                emit_window(w)
            next_img = 0
            for k in range(ncalls):
                # conv images paced evenly across the call stream
                while next_img * ncalls < (k + 1) * IMG_PER_CORE:
                    emit_conv(next_img)
                    next_img += 1
                ensure_calls(k + NQ + 2)  # keep queues fed ahead
                for w in win_of_call[k]:
                    emit_window(w)
            while next_img < IMG_PER_CORE:
                emit_conv(next_img)
                next_img += 1

            # ---- vision FC: xf = leaky(acc/WS + bias) in one Prelu
            xf = cpool.tile([D, IMG_PER_CORE * 4], bf16)
            nc.scalar.activation(out=xf[:], in_=acc_all[:], func=PR,
                                 scale=float(1.0 / WS), bias=conv_bias[:, :1],
                                 alpha=NEG)
            fcv = fc_ps.tile([IMG_PER_CORE, OUT], f32, tag="fps")
            xf3 = xf[:].rearrange("p (i q) -> p i q", q=4)
            for q in range(4):
                nc.tensor.matmul(out=fcv[:], lhsT=xf3[:, :, q],
                                 rhs=fcvW[:, q * OUT:(q + 1) * OUT],
                                 start=(q == 0), stop=False)
            nc.tensor.matmul(out=fcv[:], lhsT=ones1[:, :IMG_PER_CORE], rhs=fcv_brow[:],
                             start=False, stop=True)
            vres = smpool.tile([IMG_PER_CORE, OUT], f32, tag="vres")
            nc.scalar.activation(out=vres[:], in_=fcv[:], func=CP)
            nc.sync.dma_start(out=vis_out_d[:], in_=vres[:])

            # ---- topo FC (pooling was emitted per graph as windows completed)
            pooled_bf = smpool.tile([D, G_PER_CORE], bf16, tag="pooledb")
            nc.scalar.activation(out=pooled_bf[:], in_=pooled[:], func=CP)
            fct = fc_ps.tile([G_PER_CORE, OUT], f32, tag="fps")
            nc.tensor.matmul(out=fct[:], lhsT=pooled_bf[:], rhs=fctW[:],
                             start=True, stop=False)
            nc.tensor.matmul(out=fct[:], lhsT=ones1[:, :G_PER_CORE], rhs=fct_brow[:],
                             start=False, stop=True)
            tres = smpool.tile([G_PER_CORE, OUT], f32, tag="tres")
            nc.scalar.activation(out=tres[:], in_=fct[:], func=CP)
            nc.sync.dma_start(out=topo_out_d[:], in_=tres[:])

    nc.finalize()
    return nc


_PROG_CACHE = {}


def kernel(**inputs):
    global LAST_EXEC_NS, LAST_RESULT
    in_maps, sched = _build_schedule(inputs)
    key = (sched["t_chunks"], tuple(sched["c_w"]), tuple(sched["win_start"]),
           tuple(sched["call_sizes"]))
    if key not in _PROG_CACHE:
        _PROG_CACHE[key] = _build_program(sched["t_chunks"], sched["c_w"],
                                          sched["win_start"],
                                          sched["call_sizes"])
    nc = _PROG_CACHE[key]

    trace = os.environ.get("BASS_TRACE", "") not in ("", "0")
    res = run_bass_kernel_spmd(nc, in_maps, list(range(NCORES)), trace=trace)
    LAST_RESULT = res
    LAST_EXEC_NS = res.exec_time_ns

    vis_score = np.concatenate([res.results[c]["vis_out"] for c in range(NCORES)], axis=0)
    cg, pg = sched["core_of_graph"], sched["pos_of_graph"]
    topo_score = np.stack([res.results[int(cg[g])]["topo_out"][int(pg[g])]
                           for g in range(B)], axis=0)
    return (np.asarray(vis_score, dtype=np.float32),
            np.asarray(topo_score, dtype=np.float32))


# revision 50
# speedup vs baseline: 1.1123x; 1.1123x over previous
"""Trainium2 Bass kernel for nn_MinamoScoreHead (vision conv head + GCN topo head).

Sharding: data-parallel over 8 NeuronCores. Each core gets 8 images (vision
head) and 8 whole graphs (topo head: nodes + all edges whose dst lies in
those graphs). Small weights are replicated. Everything heavy runs on device:
 - 3x3 valid conv in fp8e4m3: taps paired into DoubleRow matmuls (2x PE rate),
   weights pre-scaled by WS to dodge fp8 subnormals; epilogue folds 1/WS +
   bias + leaky-relu into one Prelu activation.
 - adaptive max-pool via vector tensor_reduce (max commutes with the monotone
   epilogue)
 - GCN aggregation: dma_gather of (topo * dis)[src] rows by deduped
   (window, src) key over 4 SWDGE queues (the single-queue gather was the
   original bottleneck: queue serialization, not HBM bandwidth); segment-sum
   over sorted edges as matmul with fp8 count matrices S (exact small
   integers) into per-window PSUM; dis[dst] is applied post-matmul via a
   broadcast multiply, so no precision is lost to fp8 in S.
 - self-loops never touch the gather: per-window identity matmuls over
   sequentially-loaded own-node rows
 - graphs are assigned to cores by size rank so per-window chunk counts
   (max across cores) carry minimal padding
 - gcn W + bias/dis (+ padded-slot poison) via matmuls, Prelu, dis scale,
   per-graph max as windows complete
 - spectral-norm scale factors are folded into the weights on host (cheap
   O(D^2) scalar math, identical to the reference power iteration)
"""
import os
import numpy as np
import ml_dtypes

from concourse import bacc, mybir
from concourse.tile import TileContext
from concourse.bass_utils import run_bass_kernel_spmd
from concourse.ap import AP

BF16 = ml_dtypes.bfloat16
E4M3 = ml_dtypes.float8_e4m3fn

# problem constants
N_NODES = 20000
N_EDGES = 640000
D = 128
OUT = 256
B = 64
HW = 64
NEG = 0.2

NCORES = 8
IMG_PER_CORE = B // NCORES          # 8
G_PER_CORE = B // NCORES            # 8
P_G = 512                           # slots per graph
NSLOT = G_PER_CORE * P_G            # 4096
NWIN = NSLOT // 128                 # 32
CHUNK = 128                         # gather rows per scatter-matmul
CALL_CHUNKS = 8                     # chunks per dma_gather call (1024 idx: SWDGE ring cap)
NQ = 4                              # SWDGE queues (ucode max 4)
XCOLS = HW * HW + 4                 # padded image row buffer (4100)
WS = 16.0                           # conv fp8 weight pre-scale

# conv tap pairing for DoubleRow: ((dh1,dw1),(dh2,dw2)|None), rhs delta
PAIRS = [((0, 0), (1, 0)), ((0, 1), (1, 1)), ((0, 2), (1, 2)),
         ((2, 0), (2, 1)), ((2, 2), None)]
PAIR_DELTA = [64, 64, 64, 1, 1]

LAST_EXEC_NS = None
LAST_RESULT = None


def _sn_scale(w2d, u):
    """Spectral-norm 1/sigma, mimicking the reference power iteration (f32)."""
    w2d = w2d.astype(np.float32)
    u = u.astype(np.float32)
    v = w2d.T @ u
    v = v / (np.linalg.norm(v) + 1e-12)
    u2 = w2d @ v
    u2 = u2 / (np.linalg.norm(u2) + 1e-12)
    sigma = u2 @ (w2d @ v)
    return np.float32(1.0) / sigma


def _conv_blocks():
    """Row blocks of the 62-row conv output, aligned to the 31-row pool halves."""
    blocks = []
    for ph, r0 in ((0, 0), (1, 31)):
        for k in range(4):
            i0 = r0 + 8 * k
            R = 8 if k < 3 else 7
            blocks.append((i0, R, ph))
    return blocks


def _build_schedule(inputs):
    """All host-side preprocessing: shard, dedup+sort edges, build S/idx arrays."""
    vis = np.asarray(inputs["vis"], dtype=np.float32)
    topo = np.asarray(inputs["topo"], dtype=np.float32)
    edge_index = np.asarray(inputs["edge_index"], dtype=np.int64)
    batch = np.asarray(inputs["batch"], dtype=np.int64)

    conv_w = np.asarray(inputs["conv_w"], dtype=np.float32)
    conv_b = np.asarray(inputs["conv_b"], dtype=np.float32)
    fcv_w = np.asarray(inputs["fcv_w"], dtype=np.float32)
    fcv_b = np.asarray(inputs["fcv_b"], dtype=np.float32)
    gcn_w = np.asarray(inputs["gcn_w"], dtype=np.float32)
    gcn_b = np.asarray(inputs["gcn_b"], dtype=np.float32)
    fct_w = np.asarray(inputs["fct_w"], dtype=np.float32)
    fct_b = np.asarray(inputs["fct_b"], dtype=np.float32)

    # ---- spectral norm folded into weights
    s_conv = _sn_scale(conv_w.reshape(D, -1), np.asarray(inputs["conv_u"]))
    s_fcv = _sn_scale(fcv_w, np.asarray(inputs["fcv_u"]))
    s_fct = _sn_scale(fct_w, np.asarray(inputs["fct_u"]))
    conv_ws = conv_w * s_conv
    fcv_ws = fcv_w * s_fcv
    fct_ws = fct_w * s_fct

    # ---- graph structure
    counts = np.bincount(batch, minlength=B)
    assert counts.max() <= P_G, f"graph too large: {counts.max()}"
    assert counts.min() > 0, "empty graph unsupported"
    starts = np.zeros(B + 1, dtype=np.int64)
    np.cumsum(counts, out=starts[1:])
    nodes = np.arange(N_NODES, dtype=np.int64)

    # balance graphs across cores: rank by size, graph ranked[r] -> core r%8,
    # position r//8 -- so each core's position-p graph has a similar size and
    # the max-across-cores chunk padding nearly vanishes
    ranked = np.argsort(-counts, kind="stable")
    core_of_graph = np.zeros(B, dtype=np.int64)
    pos_of_graph = np.zeros(B, dtype=np.int64)
    core_of_graph[ranked] = np.arange(B) % NCORES
    pos_of_graph[ranked] = np.arange(B) // NCORES

    slot_node = pos_of_graph[batch] * P_G + (nodes - starts[batch])

    deg = (1.0 + np.bincount(edge_index[1], minlength=N_NODES)).astype(np.float32)
    dis = (1.0 / np.sqrt(deg)).astype(np.float32)

    # self-loops are handled by per-window identity matmuls, not gathered
    src_all = edge_index[0]
    dst_all = edge_index[1]
    core_all = core_of_graph[batch[dst_all]]
    dslot_all = slot_node[dst_all]

    # dis[src] folded into the gather table; dis[dst] applied post-aggregation
    table = np.ascontiguousarray((topo * dis[:, None]).astype(BF16))

    per_core = []
    win_counts = np.zeros((NCORES, NWIN), dtype=np.int64)   # unique rows per win
    for c in range(NCORES):
        sel = core_all == c
        src_c = src_all[sel]
        dslot_c = dslot_all[sel]
        win_c = dslot_c // 128
        # dedup gather rows on (window, src); S entries become edge counts
        key = win_c * N_NODES + src_c
        uk, inv = np.unique(key, return_inverse=True)
        g_win = uk // N_NODES
        g_src = uk % N_NODES
        win_counts[c] = np.bincount(g_win, minlength=NWIN)
        per_core.append((g_src, g_win, inv, dslot_c, win_c))

    c_w = (win_counts.max(axis=0) + CHUNK - 1) // CHUNK      # chunks per window
    # note: streaming windows in descending-size order (to shorten the tail)
    # was tried and crashes the neuron runtime; keep natural order
    win_chunk_base = np.zeros(NWIN, dtype=np.int64)
    win_chunk_base[1:] = np.cumsum(c_w)[:-1]
    t_chunks = int(c_w.sum())
    e_pad = t_chunks * CHUNK

    # gather calls: full-size except the tail, which uses small calls so the
    # final ring drain is short
    call_sizes = []
    rem = t_chunks
    tail_chunks = min(16, max(0, rem - CALL_CHUNKS))
    while rem > tail_chunks:
        k = min(CALL_CHUNKS, rem - tail_chunks)
        call_sizes.append(k)
        rem -= k
    while rem > 0:
        k = min(4, rem)
        call_sizes.append(k)
        rem -= k

    in_maps = []
    vis_f8 = vis.reshape(B, D, HW * HW).astype(E4M3)

    # ---- replicated weights
    # conv pairs: W2[cin, p, i, cout] = conv_ws[cout, cin, dh_i, dw_i] * WS
    W2 = np.zeros((D, len(PAIRS), 2, D), dtype=np.float32)
    for p, (t1, t2) in enumerate(PAIRS):
        for i, t in enumerate((t1, t2)):
            if t is None:
                continue
            dh, dw = t
            W2[:, p, i, :] = conv_ws[:, :, dh, dw].T * WS
    W2 = np.ascontiguousarray(W2.reshape(D, len(PAIRS) * 2 * D)).astype(E4M3)
    conv_bias = conv_b.reshape(D, 1).astype(np.float32)
    gcnW = gcn_w.astype(BF16)                                   # [d_in, d_out]
    biasp = np.stack([gcn_b, np.ones(D, np.float32)]).astype(BF16)   # [2, 128]
    # fcvW[c, q*256+o] = fcv_ws[o, c*4+q]
    fcvW = np.ascontiguousarray(
        fcv_ws.reshape(OUT, D, 4).transpose(1, 2, 0).reshape(D, 4 * OUT)).astype(BF16)
    fcv_brow = fcv_b.reshape(1, OUT).astype(BF16)
    fctW = np.ascontiguousarray(fct_ws.T).astype(BF16)          # [128, 256]
    fct_brow = fct_b.reshape(1, OUT).astype(BF16)

    I8 = np.eye(CHUNK, dtype=np.float32).astype(E4M3)

    for c in range(NCORES):
        g_src, g_win, inv, dslot_c, win_c = per_core[c]
        # row position for each unique (win, src) group: groups are sorted by
        # (win, src), so rank within window = global rank - window start rank
        ng = g_src.shape[0]
        rank = np.arange(ng, dtype=np.int64)
        gw_start = np.searchsorted(g_win, np.arange(NWIN))
        rowpos = win_chunk_base[g_win] * CHUNK + (rank - gw_start[g_win])

        srcp = np.zeros(e_pad, dtype=np.int64)
        srcp[rowpos] = g_src

        # S: [128, t_chunks*128] fp8; S[p, t*128+dcol] = count
        S = np.zeros((CHUNK, t_chunks * CHUNK), dtype=np.float32)
        erow = rowpos[inv]                       # padded row of each edge
        t_arr = erow // CHUNK
        p_arr = erow % CHUNK
        dcol = dslot_c - win_c * 128
        np.add.at(S, (p_arr, t_arr * CHUNK + dcol), 1.0)
        S8 = S.astype(E4M3)

        # gather index wrapping: idx j -> [16m + j%16, j//16], m=0..7
        base16 = srcp.reshape(-1, 16).T.astype(np.int16)         # [16, e_pad/16]
        gidx = np.ascontiguousarray(np.tile(base16, (8, 1)))     # [128, e_pad/16]

        # self-loop rows: window w holds rows [i] = table[start_g + 128*(w%4) + i]
        # (zero-padded past the graph); consumed by identity matmuls on device
        xwall = np.zeros((CHUNK, NWIN, D), dtype=BF16)
        graphs_c = ranked[np.arange(G_PER_CORE) * NCORES + c]
        for w in range(NWIN):
            gid = graphs_c[w // 4]
            r_w = int(np.clip(counts[gid] - 128 * (w % 4), 0, 128))
            if r_w > 0:
                n0 = starts[gid] + 128 * (w % 4)
                xwall[:r_w, w, :] = table[n0:n0 + r_w]

        # per-slot dis (for post-scale) and bias/poison rows
        gslots = np.arange(NSLOT, dtype=np.int64)
        gg = graphs_c[gslots // P_G]
        local = gslots % P_G
        node_of_slot = starts[gg] + local
        real = local < counts[gg]
        dis_slot = np.where(real, dis[np.minimum(node_of_slot, N_NODES - 1)], 1.0)
        dis_slot = dis_slot.astype(np.float32)
        disB = np.ascontiguousarray(
            np.broadcast_to(dis_slot.astype(BF16), (D, NSLOT)))
        maskb2 = np.zeros((2, NSLOT), dtype=np.float32)
        maskb2[0] = np.where(real, 1.0 / dis_slot, 0.0)
        maskb2[1] = np.where(real, 0.0, -1.0e9)
        maskb2 = maskb2.astype(BF16)

        in_maps.append({
            "vis": np.ascontiguousarray(vis_f8[c * IMG_PER_CORE:(c + 1) * IMG_PER_CORE]),
            "topo": table,
            "gidx": gidx,
            "S": S8,
            "XW": np.ascontiguousarray(xwall.reshape(CHUNK, NWIN * D)),
            "I8": I8,
            "disB": disB,
            "maskb2": maskb2,
            "convW2": W2,
            "conv_bias": conv_bias,
            "gcnW": gcnW,
            "biasp": biasp,
            "fcvW": fcvW,
            "fcv_brow": fcv_brow,
            "fctW": fctW,
            "fct_brow": fct_brow,
        })

    sched = dict(t_chunks=t_chunks, c_w=[int(x) for x in c_w],
                 win_start=[int(x) for x in win_chunk_base],
                 call_sizes=call_sizes,
                 core_of_graph=core_of_graph, pos_of_graph=pos_of_graph)
    return in_maps, sched


def _pair_rhs(xap, base, delta, n):
    """[128, 2, n] view of a [128, C] SBUF tile: slots at base and base+delta."""
    a = xap[:, base:base + n]
    return AP(a.tensor, a.offset, [list(a.ap[0]), [delta, 2], list(a.ap[1])])


def _build_program(t_chunks, c_w, win_start, call_sizes):
    nc = bacc.Bacc(None, target_bir_lowering=False, num_swdge_queues=NQ)
    f32 = mybir.dt.float32
    bf16 = mybir.dt.bfloat16
    fp8 = mybir.dt.float8e4

    vis_d = nc.declare_dram_parameter("vis", [IMG_PER_CORE, D, HW * HW], fp8, isOutput=False)
    topo_d = nc.declare_dram_parameter("topo", [N_NODES, D], bf16, isOutput=False)
    gidx_d = nc.declare_dram_parameter("gidx", [128, (t_chunks * CHUNK) // 16], mybir.dt.int16, isOutput=False)
    S_d = nc.declare_dram_parameter("S", [128, t_chunks * CHUNK], fp8, isOutput=False)
    XW_d = nc.declare_dram_parameter("XW", [CHUNK, NWIN * D], bf16, isOutput=False)
    I8_d = nc.declare_dram_parameter("I8", [CHUNK, CHUNK], fp8, isOutput=False)
    disB_d = nc.declare_dram_parameter("disB", [D, NSLOT], bf16, isOutput=False)
    maskb2_d = nc.declare_dram_parameter("maskb2", [2, NSLOT], bf16, isOutput=False)
    convW2_d = nc.declare_dram_parameter("convW2", [D, len(PAIRS) * 2 * D], fp8, isOutput=False)
    conv_bias_d = nc.declare_dram_parameter("conv_bias", [D, 1], f32, isOutput=False)
    gcnW_d = nc.declare_dram_parameter("gcnW", [D, D], bf16, isOutput=False)
    biasp_d = nc.declare_dram_parameter("biasp", [2, D], bf16, isOutput=False)
    fcvW_d = nc.declare_dram_parameter("fcvW", [D, 4 * OUT], bf16, isOutput=False)
    fcv_brow_d = nc.declare_dram_parameter("fcv_brow", [1, OUT], bf16, isOutput=False)
    fctW_d = nc.declare_dram_parameter("fctW", [D, OUT], bf16, isOutput=False)
    fct_brow_d = nc.declare_dram_parameter("fct_brow", [1, OUT], bf16, isOutput=False)

    vis_out_d = nc.declare_dram_parameter("vis_out", [IMG_PER_CORE, OUT], f32, isOutput=True)
    topo_out_d = nc.declare_dram_parameter("topo_out", [G_PER_CORE, OUT], f32, isOutput=True)

    ncalls = len(call_sizes)
    call_base = np.zeros(ncalls + 1, dtype=np.int64)
    np.cumsum(call_sizes, out=call_base[1:])

    # window -> last gather call it needs
    def last_call(w):
        if c_w[w] == 0:
            return -1
        last_chunk = win_start[w] + c_w[w] - 1
        return int(np.searchsorted(call_base[1:], last_chunk, side="right"))

    blocks = _conv_blocks()
    CP = mybir.ActivationFunctionType.Copy
    PR = mybir.ActivationFunctionType.Prelu
    DR = mybir.MatmulPerfMode.DoubleRow

    with TileContext(nc) as tc:
        with tc.tile_pool(name="const", bufs=1) as cpool, \
             tc.tile_pool(name="xin", bufs=2) as xpool, \
             tc.tile_pool(name="gat", bufs=14) as gpool, \
             tc.tile_pool(name="spool", bufs=14) as spool, \
             tc.tile_pool(name="small", bufs=4) as smpool, \
             tc.tile_pool(name="cps", bufs=2, space="PSUM") as conv_ps, \
             tc.tile_pool(name="aps", bufs=3, space="PSUM") as agg_ps, \
             tc.tile_pool(name="hps", bufs=2, space="PSUM") as h_ps, \
             tc.tile_pool(name="fps", bufs=1, space="PSUM") as fc_ps:

            # ---- constants (gidx head first: it gates the gather stream)
            all_cols = (t_chunks * CHUNK) // 16
            head_calls = min(NQ + 3, ncalls)
            head_cols = min(int(call_base[head_calls]) * 8, all_cols)
            gidx_h = cpool.tile([128, head_cols], mybir.dt.int16)
            nc.sync.dma_start(out=gidx_h[:], in_=gidx_d[:, :head_cols])
            gidx_r = None
            if head_cols < all_cols:
                gidx_r = cpool.tile([128, all_cols - head_cols], mybir.dt.int16)
                nc.sync.dma_start(out=gidx_r[:], in_=gidx_d[:, head_cols:])
            convW2 = cpool.tile([D, len(PAIRS), 2, D], fp8)
            conv_bias = cpool.tile([D, 1], f32)
            gcnW = cpool.tile([D, D], bf16)
            biasp = cpool.tile([2, D], bf16)
            maskb2 = cpool.tile([2, NSLOT], bf16)
            disB = cpool.tile([D, NSLOT], bf16)
            fcvW = cpool.tile([D, 4 * OUT], bf16)
            fcv_brow = cpool.tile([1, OUT], bf16)
            fctW = cpool.tile([D, OUT], bf16)
            fct_brow = cpool.tile([1, OUT], bf16)
            xwall = cpool.tile([CHUNK, NWIN, D], bf16)
            I8 = cpool.tile([CHUNK, CHUNK], fp8)

            def load_consts():
                """Emitted after the first gather calls so the small gidx head
                DMA reaches the front of the sync queue."""
                nc.sync.dma_start(out=convW2[:], in_=convW2_d[:].rearrange(
                    "c (p i o) -> c p i o", p=len(PAIRS), i=2))
                nc.sync.dma_start(out=conv_bias[:], in_=conv_bias_d[:])
                nc.sync.dma_start(out=gcnW[:], in_=gcnW_d[:])
                nc.sync.dma_start(out=biasp[:], in_=biasp_d[:])
                nc.sync.dma_start(out=maskb2[:], in_=maskb2_d[:])
                nc.sync.dma_start(out=disB[:], in_=disB_d[:])
                nc.sync.dma_start(out=fcvW[:], in_=fcvW_d[:])
                nc.sync.dma_start(out=fcv_brow[:], in_=fcv_brow_d[:])
                nc.sync.dma_start(out=fctW[:], in_=fctW_d[:])
                nc.sync.dma_start(out=fct_brow[:], in_=fct_brow_d[:])
                nc.sync.dma_start(out=xwall[:], in_=XW_d[:].rearrange(
                    "p (w d) -> p w d", w=NWIN))
                nc.sync.dma_start(out=I8[:], in_=I8_d[:])

            acc_all = cpool.tile([D, IMG_PER_CORE * 4], f32)
            nc.vector.memset(acc_all[:], -3.0e38)
            ones1 = cpool.tile([1, max(IMG_PER_CORE, G_PER_CORE)], bf16)
            nc.vector.memset(ones1[:], 1.0)
            aggT = cpool.tile([D, NSLOT], bf16)
            hT = cpool.tile([D, NSLOT], bf16)

            gtiles = {}
            stiles = {}
            pooled = cpool.tile([D, G_PER_CORE], f32)
            wins_done = [0] * G_PER_CORE

            def note_window_done(w):
                gidx_ = w // 4
                wins_done[gidx_] += 1
                if wins_done[gidx_] == 4:
                    nc.vector.tensor_reduce(
                        out=pooled[:, gidx_:gidx_ + 1],
                        in_=hT[:, gidx_ * P_G:(gidx_ + 1) * P_G],
                        axis=mybir.AxisListType.X, op=mybir.AluOpType.max)

            def emit_call(k):
                nchunk = call_sizes[k]
                nidx = nchunk * CHUNK
                g = gpool.tile([128, CALL_CHUNKS, CHUNK], bf16, tag="gat")
                c0, c1 = int(call_base[k]) * 8, int(call_base[k + 1]) * 8
                if k < head_calls:
                    idxs = gidx_h[:, c0:c1]
                else:
                    idxs = gidx_r[:, c0 - head_cols:c1 - head_cols]
                nc.gpsimd.dma_gather(
                    out_ap=g[:, :nchunk, :],
                    in_ap=topo_d[:],
                    idxs_ap=idxs,
                    num_idxs=nidx,
                    num_idxs_reg=nidx,
                    elem_size=D,
                    queue_num=k % NQ,
                )
                s = spool.tile([128, CALL_CHUNKS * CHUNK], fp8, tag="spool")
                eng = nc.sync
                eng.dma_start(
                    out=s[:, :nchunk * CHUNK],
                    in_=S_d[:, int(call_base[k]) * CHUNK: int(call_base[k + 1]) * CHUNK],
                )
                gtiles[k] = g
                stiles[k] = s

            def emit_window(w):
                agg = agg_ps.tile([D, 128], f32, tag="aps")
                # self-loops: agg[:, slot] += XW[slot, :] (zero rows past graph)
                nc.tensor.matmul(out=agg[:], lhsT=xwall[:, w, :], rhs=I8[:],
                                 start=True, stop=(c_w[w] == 0))
                wend = win_start[w] + c_w[w]
                for t in range(int(win_start[w]), int(wend)):
                    k = int(np.searchsorted(call_base[1:], t, side="right"))
                    off = t - int(call_base[k])
                    nc.tensor.matmul(
                        out=agg[:],
                        lhsT=gtiles[k][:, off, :],
                        rhs=stiles[k][:, off * CHUNK:(off + 1) * CHUNK],
                        start=False, stop=(t == int(wend) - 1),
                    )
                nc.scalar.activation(out=aggT[:, w * 128:(w + 1) * 128], in_=agg[:], func=CP)
                h = h_ps.tile([D, 128], f32, tag="hps")
                nc.tensor.matmul(out=h[:], lhsT=gcnW[:], rhs=aggT[:, w * 128:(w + 1) * 128],
                                 start=True, stop=False)
                nc.tensor.matmul(out=h[:], lhsT=biasp[:], rhs=maskb2[:, w * 128:(w + 1) * 128],
                                 start=False, stop=True)
                # leaky(h) then * dis[dst]  (leaky(x*d) = leaky(x)*d for d>0)
                hl = smpool.tile([D, 128], bf16, tag="hl")
                nc.scalar.activation(out=hl[:], in_=h[:], func=PR, alpha=NEG)
                nc.vector.tensor_tensor(out=hT[:, w * 128:(w + 1) * 128],
                                        in0=hl[:], in1=disB[:, w * 128:(w + 1) * 128],
                                        op=mybir.AluOpType.mult)
                note_window_done(w)

            def emit_conv(img):
                x = xpool.tile([D, XCOLS], fp8, tag="xin")
                nc.sync.dma_start(out=x[:, :HW * HW], in_=vis_d[img])
                nc.vector.memset(x[:, HW * HW:], 0.0)
                xap = x[:]
                for (i0, R, ph2) in blocks:
                    n = R * HW
                    ps = conv_ps.tile([D, 512], f32, tag="cps")
                    for p in range(len(PAIRS)):
                        (dh1, dw1) = PAIRS[p][0]
                        base = (i0 + dh1) * HW + dw1
                        nc.tensor.matmul(
                            out=ps[:, :n],
                            lhsT=convW2[:, p, :, :],
                            rhs=_pair_rhs(xap, base, PAIR_DELTA[p], n),
                            start=(p == 0), stop=(p == len(PAIRS) - 1),
                            perf_mode=DR,
                        )
                    red = smpool.tile([D, 2], f32, tag="red")
                    ap = ps[:, :n].rearrange("p (r c) -> p r c", c=HW)[:, :, :62] \
                                  .rearrange("p r (q w) -> p q r w", q=2)
                    nc.vector.tensor_reduce(out=red[:], in_=ap, axis=mybir.AxisListType.XY,
                                            op=mybir.AluOpType.max)
                    accs = acc_all[:, img * 4 + ph2 * 2: img * 4 + ph2 * 2 + 2]
                    nc.vector.tensor_tensor(out=accs, in0=accs, in1=red[:],
                                            op=mybir.AluOpType.max)

            # ---- emission schedule: interleave conv images, gather calls, windows
            win_of_call = [[] for _ in range(ncalls)]
            for w in range(NWIN):
                lc = last_call(w)
                if lc >= 0:
                    win_of_call[lc].append(w)
            empty_wins = [w for w in range(NWIN) if c_w[w] == 0]

            emitted_calls = 0

            def ensure_calls(upto):
                nonlocal emitted_calls
                while emitted_calls <= min(upto, ncalls - 1):
                    emit_call(emitted_calls)
                    emitted_calls += 1

            load_consts()
            ensure_calls(NQ + 2)     # prefetch first calls
            for w in empty_wins:
                emit_window(w)
            next_img = 0
            for k in range(ncalls):
                # conv images paced evenly across the call stream
                while next_img * ncalls < (k + 1) * IMG_PER_CORE:
                    emit_conv(next_img)
                    next_img += 1
                ensure_calls(k + NQ + 2)  # keep queues fed ahead
                for w in win_of_call[k]:
                    emit_window(w)
            while next_img < IMG_PER_CORE:
                emit_conv(next_img)
                next_img += 1

            # ---- vision FC: xf = leaky(acc/WS + bias) in one Prelu
            xf = cpool.tile([D, IMG_PER_CORE * 4], bf16)
            nc.scalar.activation(out=xf[:], in_=acc_all[:], func=PR,
                                 scale=float(1.0 / WS), bias=conv_bias[:, :1],
                                 alpha=NEG)
            fcv = fc_ps.tile([IMG_PER_CORE, OUT], f32, tag="fps")
            xf3 = xf[:].rearrange("p (i q) -> p i q", q=4)
            for q in range(4):
                nc.tensor.matmul(out=fcv[:], lhsT=xf3[:, :, q],
                                 rhs=fcvW[:, q * OUT:(q + 1) * OUT],
                                 start=(q == 0), stop=False)
            nc.tensor.matmul(out=fcv[:], lhsT=ones1[:, :IMG_PER_CORE], rhs=fcv_brow[:],
                             start=False, stop=True)
            vres = smpool.tile([IMG_PER_CORE, OUT], f32, tag="vres")
            nc.scalar.activation(out=vres[:], in_=fcv[:], func=CP)
            nc.sync.dma_start(out=vis_out_d[:], in_=vres[:])

            # ---- topo FC (pooling was emitted per graph as windows completed)
            pooled_bf = smpool.tile([D, G_PER_CORE], bf16, tag="pooledb")
            nc.scalar.activation(out=pooled_bf[:], in_=pooled[:], func=CP)
            fct = fc_ps.tile([G_PER_CORE, OUT], f32, tag="fps")
            nc.tensor.matmul(out=fct[:], lhsT=pooled_bf[:], rhs=fctW[:],
                             start=True, stop=False)
            nc.tensor.matmul(out=fct[:], lhsT=ones1[:, :G_PER_CORE], rhs=fct_brow[:],
                             start=False, stop=True)
            tres = smpool.tile([G_PER_CORE, OUT], f32, tag="tres")
            nc.scalar.activation(out=tres[:], in_=fct[:], func=CP)
            nc.sync.dma_start(out=topo_out_d[:], in_=tres[:])

    nc.finalize()
    return nc


_PROG_CACHE = {}


def kernel(**inputs):
    global LAST_EXEC_NS, LAST_RESULT
    in_maps, sched = _build_schedule(inputs)
    key = (sched["t_chunks"], tuple(sched["c_w"]), tuple(sched["win_start"]),
           tuple(sched["call_sizes"]))
    if key not in _PROG_CACHE:
        _PROG_CACHE[key] = _build_program(sched["t_chunks"], sched["c_w"],
                                          sched["win_start"],
                                          sched["call_sizes"])
    nc = _PROG_CACHE[key]

    trace = os.environ.get("BASS_TRACE", "") not in ("", "0")
    res = run_bass_kernel_spmd(nc, in_maps, list(range(NCORES)), trace=trace)
    LAST_RESULT = res
    LAST_EXEC_NS = res.exec_time_ns

    vis_score = np.concatenate([res.results[c]["vis_out"] for c in range(NCORES)], axis=0)
    cg, pg = sched["core_of_graph"], sched["pos_of_graph"]
    topo_score = np.stack([res.results[int(cg[g])]["topo_out"][int(pg[g])]
                           for g in range(B)], axis=0)
    return (np.asarray(vis_score, dtype=np.float32),
            np.asarray(topo_score, dtype=np.float32))
